# revision 2
# baseline (speedup 1.0000x reference)
"""MoE layer (16 experts, top-4, silu-gated FFN + shared expert) on 8 trn2 cores.

Strategy (expert-parallel, host-side dispatch):
  - Host computes the router (softmax + top-4 + renormalize) in numpy —
    0.2% of total FLOPs — and gathers each expert's tokens into a padded
    [capacity] batch (classic MoE dispatch, done host-side instead of
    device all-to-all).
  - Each of the 8 cores holds 2 experts (weights resident in SBUF, bf16)
    and runs the dense silu-gated FFN over its experts' gathered tokens,
    scaling activations by the combine weights before the down-projection
    so partial outputs can be scatter-added on the host.
  - The shared expert is data-parallel: core i handles tokens
    [i*256, (i+1)*256).
  - All activations/weights are bf16 (PE: 1 cycle/row vs 2 for fp32),
    accumulation in fp32 PSUM; partial outputs returned fp32.

Device layout: activations kept transposed ([d_model, tokens]: feature on
partitions, tokens on the free dim) so both matmuls feed the PE without any
on-device transpose; combine weights arrive pre-broadcast as [128, C] rows.
"""

import os
import numpy as np
import ml_dtypes

DIM = 1024
HID = 512
E = 16
TOPK = 4
NCORES = 8
EPC = E // NCORES  # experts per core
T = 2048
S = T // NCORES  # shared-expert tokens per core

BF16 = ml_dtypes.bfloat16

_CACHE = {}


def _build(C: int):
    """Build + schedule the SPMD Tile kernel for per-expert capacity C."""
    import concourse.bass as bass
    import concourse.tile as tile
    import concourse.mybir as mybir
    from concourse import bacc

    f32 = mybir.dt.float32
    bf16 = mybir.dt.bfloat16

    nc = bacc.Bacc("TRN2", target_bir_lowering=False, debug=False,
                   num_devices=NCORES)

    xe = nc.dram_tensor("xe", [EPC, DIM, C], bf16, kind="ExternalInput")
    cb = nc.dram_tensor("cb", [EPC, 128, C], f32, kind="ExternalInput")
    we1 = nc.dram_tensor("we1", [EPC, DIM, HID], bf16, kind="ExternalInput")
    we3 = nc.dram_tensor("we3", [EPC, DIM, HID], bf16, kind="ExternalInput")
    we2 = nc.dram_tensor("we2", [EPC, HID, DIM], bf16, kind="ExternalInput")
    xs = nc.dram_tensor("xs", [DIM, S], bf16, kind="ExternalInput")
    ws1 = nc.dram_tensor("ws1", [DIM, HID], bf16, kind="ExternalInput")
    ws3 = nc.dram_tensor("ws3", [DIM, HID], bf16, kind="ExternalInput")
    ws2 = nc.dram_tensor("ws2", [HID, DIM], bf16, kind="ExternalInput")
    oute = nc.dram_tensor("oute", [EPC, DIM, C], f32, kind="ExternalOutput")
    outs = nc.dram_tensor("outs", [DIM, S], f32, kind="ExternalOutput")

    DK = DIM // 128   # 8 contraction tiles for the up-projections
    HK = HID // 128   # 4 contraction tiles for the down-projection

    def chunks(total):
        out, n0 = [], 0
        while n0 < total:
            n = min(512, total - n0)
            out.append((n0, n))
            n0 += n
        return out

    with tile.TileContext(nc) as tc:
        with (
            tc.tile_pool(name="wts", bufs=1) as wts,
            tc.tile_pool(name="acts", bufs=1) as actp,
            tc.tile_pool(name="work", bufs=2) as work,
            tc.tile_pool(name="ost", bufs=4) as ostp,
            tc.tile_pool(name="ph", bufs=2, space="PSUM") as ph,
            tc.tile_pool(name="po", bufs=2, space="PSUM") as po,
        ):
            # jobs: (w1 tiles, w3 tiles, w2 tiles, x tiles, cb tile | None,
            #        n_tokens, out dram, label)
            jobs = []
            for e in range(EPC):
                w1_t = [wts.tile([128, HID], bf16, tag=f"w1_{e}_{k}", name=f"w1_{e}_{k}") for k in range(DK)]
                w3_t = [wts.tile([128, HID], bf16, tag=f"w3_{e}_{k}", name=f"w3_{e}_{k}") for k in range(DK)]
                w2_t = [wts.tile([128, DIM], bf16, tag=f"w2_{e}_{k}", name=f"w2_{e}_{k}") for k in range(HK)]
                x_t = [actp.tile([128, C], bf16, tag=f"xe_{e}_{k}", name=f"xe_{e}_{k}") for k in range(DK)]
                cb_t = actp.tile([128, C], f32, tag=f"cb_{e}", name=f"cbt_{e}")
                for k in range(DK):
                    nc.sync.dma_start(out=w1_t[k][:], in_=we1[e, k * 128:(k + 1) * 128, :])
                    nc.sync.dma_start(out=w3_t[k][:], in_=we3[e, k * 128:(k + 1) * 128, :])
                    nc.sync.dma_start(out=x_t[k][:], in_=xe[e, k * 128:(k + 1) * 128, :])
                for k in range(HK):
                    nc.sync.dma_start(out=w2_t[k][:], in_=we2[e, k * 128:(k + 1) * 128, :])
                nc.sync.dma_start(out=cb_t[:], in_=cb[e])
                jobs.append((w1_t, w3_t, w2_t, x_t, cb_t, C, oute, e))
            # shared expert job
            w1_t = [wts.tile([128, HID], bf16, tag=f"sw1_{k}", name=f"sw1_{k}") for k in range(DK)]
            w3_t = [wts.tile([128, HID], bf16, tag=f"sw3_{k}", name=f"sw3_{k}") for k in range(DK)]
            w2_t = [wts.tile([128, DIM], bf16, tag=f"sw2_{k}", name=f"sw2_{k}") for k in range(HK)]
            x_t = [actp.tile([128, S], bf16, tag=f"xs_{k}", name=f"xst_{k}") for k in range(DK)]
            for k in range(DK):
                nc.sync.dma_start(out=w1_t[k][:], in_=ws1[k * 128:(k + 1) * 128, :])
                nc.sync.dma_start(out=w3_t[k][:], in_=ws3[k * 128:(k + 1) * 128, :])
                nc.sync.dma_start(out=x_t[k][:], in_=xs[k * 128:(k + 1) * 128, :])
            for k in range(HK):
                nc.sync.dma_start(out=w2_t[k][:], in_=ws2[k * 128:(k + 1) * 128, :])
            jobs.append((w1_t, w3_t, w2_t, x_t, None, S, outs, None))

            for (w1_t, w3_t, w2_t, x_t, cb_t, ntok, odram, e) in jobs:
                for (n0, n) in chunks(ntok):
                    act_t = []
                    for hm in range(HK):
                        hsl = slice(hm * 128, (hm + 1) * 128)
                        p1 = ph.tile([128, 512], f32, tag="h1", name="p1")
                        p3 = ph.tile([128, 512], f32, tag="h3", name="p3")
                        for k in range(DK):
                            nc.tensor.matmul(p1[:, :n], w1_t[k][:, hsl],
                                             x_t[k][:, n0:n0 + n],
                                             start=(k == 0), stop=(k == DK - 1))
                        for k in range(DK):
                            nc.tensor.matmul(p3[:, :n], w3_t[k][:, hsl],
                                             x_t[k][:, n0:n0 + n],
                                             start=(k == 0), stop=(k == DK - 1))
                        sil = work.tile([128, 512], bf16, tag="sil", name="sil")
                        nc.scalar.activation(sil[:, :n], p1[:, :n],
                                             mybir.ActivationFunctionType.Silu)
                        a = work.tile([128, 512], bf16, tag=f"act{hm}", name=f"act{hm}")
                        if cb_t is not None:
                            h3s = work.tile([128, 512], bf16, tag="h3s", name="h3s")
                            nc.vector.tensor_tensor(h3s[:, :n], p3[:, :n],
                                                    cb_t[:, n0:n0 + n],
                                                    mybir.AluOpType.mult)
                            nc.vector.tensor_tensor(a[:, :n], h3s[:, :n],
                                                    sil[:, :n],
                                                    mybir.AluOpType.mult)
                        else:
                            nc.vector.tensor_tensor(a[:, :n], p3[:, :n],
                                                    sil[:, :n],
                                                    mybir.AluOpType.mult)
                        act_t.append(a)
                    for dm in range(DK):
                        dsl = slice(dm * 128, (dm + 1) * 128)
                        pout = po.tile([128, 512], f32, tag="o", name="pout")
                        for k in range(HK):
                            nc.tensor.matmul(pout[:, :n], w2_t[k][:, dsl],
                                             act_t[k][:, :n],
                                             start=(k == 0), stop=(k == HK - 1))
                        ob = ostp.tile([128, 512], f32, tag="ob", name="ob")
                        nc.vector.tensor_copy(out=ob[:, :n], in_=pout[:, :n])
                        if e is None:
                            dst = odram[dsl, n0:n0 + n]
                        else:
                            dst = odram[e, dsl, n0:n0 + n]
                        nc.sync.dma_start(out=dst, in_=ob[:, :n])

    nc.compile()
    return nc


def _get_nc(C: int):
    if C not in _CACHE:
        _CACHE[C] = _build(C)
    return _CACHE[C]


LAST_RESULTS = None  # BassKernelResults from the most recent run (for test.py)


def kernel(x, gate_w, w1, w3, w2, sw1, sw3, sw2):
    global LAST_RESULTS
    from concourse.bass_utils import run_bass_kernel_spmd

    x = np.asarray(x)
    xf = np.ascontiguousarray(x.reshape(-1, DIM).astype(np.float32))
    gate_w = np.asarray(gate_w, dtype=np.float32)

    # ---- router on host (softmax -> top-4 -> renormalize) ----
    logits = xf @ gate_w.T                      # [T, E]
    m = logits.max(axis=1, keepdims=True)
    p = np.exp(logits - m)
    probs = p / p.sum(axis=1, keepdims=True)
    idx4 = np.argpartition(-probs, TOPK, axis=1)[:, :TOPK]     # [T, 4]
    w4 = np.take_along_axis(probs, idx4, axis=1)
    w4 = w4 / w4.sum(axis=1, keepdims=True)

    rows = np.repeat(np.arange(xf.shape[0]), TOPK)
    cols = idx4.ravel()
    vals = w4.ravel()

    tok_of = [rows[cols == e] for e in range(E)]
    cw_of = [vals[cols == e].astype(np.float32) for e in range(E)]
    counts = np.array([len(t) for t in tok_of])
    C = int(max(512, -(-counts.max() // 64) * 64))

    xf_bf = xf.astype(BF16)
    w1 = np.asarray(w1, dtype=np.float32)
    w3 = np.asarray(w3, dtype=np.float32)
    w2 = np.asarray(w2, dtype=np.float32)

    in_maps = []
    for c in range(NCORES):
        es = [c * EPC + j for j in range(EPC)]
        xe_np = np.zeros((EPC, DIM, C), dtype=BF16)
        cb_np = np.zeros((EPC, 128, C), dtype=np.float32)
        for j, e in enumerate(es):
            cnt = counts[e]
            xe_np[j, :, :cnt] = xf_bf[tok_of[e]].T
            cb_np[j, :, :cnt] = cw_of[e][None, :]
        im = {
            "xe": xe_np,
            "cb": cb_np,
            "we1": np.ascontiguousarray(
                w1[es].transpose(0, 2, 1)).astype(BF16),
            "we3": np.ascontiguousarray(
                w3[es].transpose(0, 2, 1)).astype(BF16),
            "we2": np.ascontiguousarray(
                w2[es].transpose(0, 2, 1)).astype(BF16),
            "xs": np.ascontiguousarray(xf_bf[c * S:(c + 1) * S].T),
            "ws1": np.ascontiguousarray(np.asarray(sw1, np.float32).T).astype(BF16),
            "ws3": np.ascontiguousarray(np.asarray(sw3, np.float32).T).astype(BF16),
            "ws2": np.ascontiguousarray(np.asarray(sw2, np.float32).T).astype(BF16),
        }
        in_maps.append(im)

    nc = _get_nc(C)
    trace = os.environ.get("KERNEL_TRACE", "0") == "1"
    res = run_bass_kernel_spmd(nc, in_maps, core_ids=list(range(NCORES)),
                               trace=trace)
    LAST_RESULTS = res

    out = np.zeros((T, DIM), dtype=np.float32)
    for c in range(NCORES):
        r = res.results[c]
        for j in range(EPC):
            e = c * EPC + j
            cnt = counts[e]
            out[tok_of[e]] += r["oute"][j, :, :cnt].T
        out[c * S:(c + 1) * S] += r["outs"].T
    return out.reshape(x.shape).astype(np.float32)


# revision 3
# speedup vs baseline: 1.1889x; 1.1889x over previous
"""MoE layer (16 experts, top-4, silu-gated FFN + shared expert) on 8 trn2 cores.

Strategy (expert-parallel, host-side dispatch):
  - Host computes the router (softmax + top-4 + renormalize) in numpy —
    0.2% of total FLOPs — and gathers each expert's tokens into a padded
    [capacity] batch (classic MoE dispatch, done host-side instead of
    device all-to-all).
  - Each of the 8 cores holds 2 experts (weights resident in SBUF, bf16)
    and runs the dense silu-gated FFN over its experts' gathered tokens,
    scaling activations by the combine weights before the down-projection
    so partial outputs can be scatter-added on the host.
  - The shared expert is data-parallel: core i handles tokens
    [i*256, (i+1)*256).
  - All activations/weights are bf16 (PE: 1 cycle/row vs 2 for fp32),
    accumulation in fp32 PSUM; partial outputs returned fp32.

Device layout: activations kept transposed ([d_model, tokens]: feature on
partitions, tokens on the free dim) so both matmuls feed the PE without any
on-device transpose; combine weights arrive pre-broadcast as [128, C] rows.
DMA is batched: one dma_start per tensor per expert (3D SBUF tiles
[128, k, f] via "(k p) f -> p k f" APs) — per-dma_start fixed cost on the
sync engine was the first-profile bottleneck. Token chunks are ordered so
the <512-token tails run last (small store tail).
"""

import os
import numpy as np
import ml_dtypes

DIM = 1024
HID = 512
E = 16
TOPK = 4
NCORES = 8
EPC = E // NCORES  # experts per core
T = 2048
S = T // NCORES  # shared-expert tokens per core

BF16 = ml_dtypes.bfloat16

_CACHE = {}


def _build(C: int):
    """Build + schedule the SPMD Tile kernel for per-expert capacity C."""
    import concourse.tile as tile
    import concourse.mybir as mybir
    from concourse import bacc

    f32 = mybir.dt.float32
    bf16 = mybir.dt.bfloat16

    nc = bacc.Bacc("TRN2", target_bir_lowering=False, debug=False,
                   num_devices=NCORES)

    xe = nc.dram_tensor("xe", [EPC, DIM, C], bf16, kind="ExternalInput")
    cb = nc.dram_tensor("cb", [EPC, 128, C], f32, kind="ExternalInput")
    we1 = nc.dram_tensor("we1", [EPC, DIM, HID], bf16, kind="ExternalInput")
    we3 = nc.dram_tensor("we3", [EPC, DIM, HID], bf16, kind="ExternalInput")
    we2 = nc.dram_tensor("we2", [EPC, HID, DIM], bf16, kind="ExternalInput")
    xs = nc.dram_tensor("xs", [DIM, S], bf16, kind="ExternalInput")
    ws1 = nc.dram_tensor("ws1", [DIM, HID], bf16, kind="ExternalInput")
    ws3 = nc.dram_tensor("ws3", [DIM, HID], bf16, kind="ExternalInput")
    ws2 = nc.dram_tensor("ws2", [HID, DIM], bf16, kind="ExternalInput")
    oute = nc.dram_tensor("oute", [EPC, DIM, C], f32, kind="ExternalOutput")
    outs = nc.dram_tensor("outs", [DIM, S], f32, kind="ExternalOutput")

    DK = DIM // 128   # 8 contraction tiles for the up-projections
    HK = HID // 128   # 4 contraction tiles for the down-projection

    def as_pkf(ap):
        return ap.rearrange("(k p) f -> p k f", p=128)

    with tile.TileContext(nc) as tc:
        with (
            tc.tile_pool(name="wts", bufs=1) as wts,
            tc.tile_pool(name="acts", bufs=1) as actp,
            tc.tile_pool(name="work", bufs=2) as work,
            tc.tile_pool(name="ost", bufs=2) as ostp,
            tc.tile_pool(name="ph", bufs=2, space="PSUM") as ph,
            tc.tile_pool(name="po", bufs=2, space="PSUM") as po,
        ):
            jobs = []
            for e in range(EPC):
                w1_t = wts.tile([128, DK, HID], bf16, name=f"w1_{e}")
                w3_t = wts.tile([128, DK, HID], bf16, name=f"w3_{e}")
                w2_t = wts.tile([128, HK, DIM], bf16, name=f"w2_{e}")
                x_t = actp.tile([128, DK, C], bf16, name=f"xe_{e}")
                cb_t = actp.tile([128, C], f32, name=f"cbt_{e}")
                nc.sync.dma_start(out=w1_t[:], in_=as_pkf(we1[e]))
                nc.sync.dma_start(out=w3_t[:], in_=as_pkf(we3[e]))
                nc.sync.dma_start(out=w2_t[:], in_=as_pkf(we2[e]))
                nc.sync.dma_start(out=x_t[:], in_=as_pkf(xe[e]))
                nc.sync.dma_start(out=cb_t[:], in_=cb[e])
                jobs.append((w1_t, w3_t, w2_t, x_t, cb_t,
                             as_pkf(oute[e])))
            w1_s = wts.tile([128, DK, HID], bf16, name="sw1")
            w3_s = wts.tile([128, DK, HID], bf16, name="sw3")
            w2_s = wts.tile([128, HK, DIM], bf16, name="sw2")
            x_s = actp.tile([128, DK, S], bf16, name="xst")
            nc.sync.dma_start(out=w1_s[:], in_=as_pkf(ws1[:]))
            nc.sync.dma_start(out=w3_s[:], in_=as_pkf(ws3[:]))
            nc.sync.dma_start(out=w2_s[:], in_=as_pkf(ws2[:]))
            nc.sync.dma_start(out=x_s[:], in_=as_pkf(xs[:]))
            jobs.append((w1_s, w3_s, w2_s, x_s, None, as_pkf(outs[:])))

            # chunk worklist: full 512-token chunks first, tails last
            work_items = []
            tails = []
            for j, job in enumerate(jobs):
                ntok = S if job[4] is None else C
                n0 = 0
                while n0 < ntok:
                    n = min(512, ntok - n0)
                    (work_items if n == 512 or j == EPC else tails).append(
                        (job, n0, n))
                    n0 += n
            work_items += tails

            for (job, n0, n) in work_items:
                w1_t, w3_t, w2_t, x_t, cb_t, o_ap = job
                act_t = []
                for hm in range(HK):
                    hsl = slice(hm * 128, (hm + 1) * 128)
                    p1 = ph.tile([128, 512], f32, tag="h1", name="p1")
                    p3 = ph.tile([128, 512], f32, tag="h3", name="p3")
                    for k in range(DK):
                        nc.tensor.matmul(p1[:, :n], w1_t[:, k, hsl],
                                         x_t[:, k, n0:n0 + n],
                                         start=(k == 0), stop=(k == DK - 1))
                    for k in range(DK):
                        nc.tensor.matmul(p3[:, :n], w3_t[:, k, hsl],
                                         x_t[:, k, n0:n0 + n],
                                         start=(k == 0), stop=(k == DK - 1))
                    sil = work.tile([128, 512], bf16, tag="sil", name="sil")
                    nc.scalar.activation(sil[:, :n], p1[:, :n],
                                         mybir.ActivationFunctionType.Silu)
                    a = work.tile([128, 512], bf16, tag=f"act{hm}",
                                  name=f"act{hm}")
                    if cb_t is not None:
                        h3s = work.tile([128, 512], bf16, tag="h3s",
                                        name="h3s")
                        nc.vector.tensor_tensor(h3s[:, :n], p3[:, :n],
                                                cb_t[:, n0:n0 + n],
                                                mybir.AluOpType.mult)
                        nc.vector.tensor_tensor(a[:, :n], h3s[:, :n],
                                                sil[:, :n],
                                                mybir.AluOpType.mult)
                    else:
                        nc.vector.tensor_tensor(a[:, :n], p3[:, :n],
                                                sil[:, :n],
                                                mybir.AluOpType.mult)
                    act_t.append(a)
                stage = ostp.tile([128, DK, 512], f32, tag="stage",
                                  name="stage")
                for dm in range(DK):
                    dsl = slice(dm * 128, (dm + 1) * 128)
                    pout = po.tile([128, 512], f32, tag="o", name="pout")
                    for k in range(HK):
                        nc.tensor.matmul(pout[:, :n], w2_t[:, k, dsl],
                                         act_t[k][:, :n],
                                         start=(k == 0), stop=(k == HK - 1))
                    nc.vector.tensor_copy(out=stage[:, dm, :n],
                                          in_=pout[:, :n])
                nc.sync.dma_start(out=o_ap[:, :, n0:n0 + n],
                                  in_=stage[:, :, :n])

    nc.compile()
    return nc


def _get_nc(C: int):
    if C not in _CACHE:
        _CACHE[C] = _build(C)
    return _CACHE[C]


LAST_RESULTS = None  # BassKernelResults from the most recent run (for test.py)


def kernel(x, gate_w, w1, w3, w2, sw1, sw3, sw2):
    global LAST_RESULTS
    from concourse.bass_utils import run_bass_kernel_spmd

    x = np.asarray(x)
    xf = np.ascontiguousarray(x.reshape(-1, DIM).astype(np.float32))
    gate_w = np.asarray(gate_w, dtype=np.float32)

    # ---- router on host (softmax -> top-4 -> renormalize) ----
    logits = xf @ gate_w.T                      # [T, E]
    m = logits.max(axis=1, keepdims=True)
    p = np.exp(logits - m)
    probs = p / p.sum(axis=1, keepdims=True)
    idx4 = np.argpartition(-probs, TOPK, axis=1)[:, :TOPK]     # [T, 4]
    w4 = np.take_along_axis(probs, idx4, axis=1)
    w4 = w4 / w4.sum(axis=1, keepdims=True)

    rows = np.repeat(np.arange(xf.shape[0]), TOPK)
    cols = idx4.ravel()
    vals = w4.ravel()

    tok_of = [rows[cols == e] for e in range(E)]
    cw_of = [vals[cols == e].astype(np.float32) for e in range(E)]
    counts = np.array([len(t) for t in tok_of])
    C = int(max(512, -(-counts.max() // 64) * 64))

    xf_bf = xf.astype(BF16)
    w1 = np.asarray(w1, dtype=np.float32)
    w3 = np.asarray(w3, dtype=np.float32)
    w2 = np.asarray(w2, dtype=np.float32)

    in_maps = []
    for c in range(NCORES):
        es = [c * EPC + j for j in range(EPC)]
        xe_np = np.zeros((EPC, DIM, C), dtype=BF16)
        cb_np = np.zeros((EPC, 128, C), dtype=np.float32)
        for j, e in enumerate(es):
            cnt = counts[e]
            xe_np[j, :, :cnt] = xf_bf[tok_of[e]].T
            cb_np[j, :, :cnt] = cw_of[e][None, :]
        im = {
            "xe": xe_np,
            "cb": cb_np,
            "we1": np.ascontiguousarray(
                w1[es].transpose(0, 2, 1)).astype(BF16),
            "we3": np.ascontiguousarray(
                w3[es].transpose(0, 2, 1)).astype(BF16),
            "we2": np.ascontiguousarray(
                w2[es].transpose(0, 2, 1)).astype(BF16),
            "xs": np.ascontiguousarray(xf_bf[c * S:(c + 1) * S].T),
            "ws1": np.ascontiguousarray(np.asarray(sw1, np.float32).T).astype(BF16),
            "ws3": np.ascontiguousarray(np.asarray(sw3, np.float32).T).astype(BF16),
            "ws2": np.ascontiguousarray(np.asarray(sw2, np.float32).T).astype(BF16),
        }
        in_maps.append(im)

    nc = _get_nc(C)
    trace = os.environ.get("KERNEL_TRACE", "0") == "1"
    res = run_bass_kernel_spmd(nc, in_maps, core_ids=list(range(NCORES)),
                               trace=trace)
    LAST_RESULTS = res

    out = np.zeros((T, DIM), dtype=np.float32)
    for c in range(NCORES):
        r = res.results[c]
        for j in range(EPC):
            e = c * EPC + j
            cnt = counts[e]
            out[tok_of[e]] += r["oute"][j, :, :cnt].T
        out[c * S:(c + 1) * S] += r["outs"].T
    return out.reshape(x.shape).astype(np.float32)


# revision 4
# speedup vs baseline: 1.3452x; 1.1314x over previous
"""MoE layer (16 experts, top-4, silu-gated FFN + shared expert) on 8 trn2 cores.

Strategy (expert-parallel, host-side dispatch):
  - Host computes the router (softmax + top-4 + renormalize) in numpy —
    0.2% of total FLOPs — and gathers each expert's tokens into a padded
    [capacity] batch (classic MoE dispatch, done host-side instead of
    device all-to-all).
  - Each of the 8 cores holds 2 experts (weights resident in SBUF, bf16)
    and runs the dense silu-gated FFN over its experts' gathered tokens,
    scaling activations by the combine weights before the down-projection
    so partial outputs can be scatter-added on the host.
  - The shared expert is data-parallel: core i handles tokens
    [i*256, (i+1)*256).
  - All activations/weights are bf16 (PE: 1 cycle/row vs 2 for fp32),
    accumulation in fp32 PSUM.

Device layout: activations kept transposed ([d_model, tokens]: feature on
partitions, tokens on the free dim) so both matmuls feed the PE without any
on-device transpose; combine weights arrive pre-broadcast as [128, C] rows.
DMA is batched (one dma_start per tensor) except expert 0's first-needed
tensors, which are split in k-halves so the PE can start ~4us in. Token
chunks are equal halves (e.g. 288+288 for C=576) so no chunk is so short
that LDWEIGHTS dominates.
"""

import os
import numpy as np
import ml_dtypes

DIM = 1024
HID = 512
E = 16
TOPK = 4
NCORES = 8
EPC = E // NCORES  # experts per core
T = 2048
S = T // NCORES  # shared-expert tokens per core

BF16 = ml_dtypes.bfloat16
OUT_BF16 = os.environ.get("KERNEL_OUT_F32", "0") != "1"

_CACHE = {}


def _chunks(total):
    if total <= 512:
        return [(0, total)]
    nch = -(-total // 512)
    base = -(-total // (nch * 16)) * 16
    out, n0 = [], 0
    while n0 < total:
        n = min(base, total - n0)
        out.append((n0, n))
        n0 += n
    return out


def _build(C: int):
    """Build + schedule the SPMD Tile kernel for per-expert capacity C."""
    import concourse.tile as tile
    import concourse.mybir as mybir
    from concourse import bacc

    f32 = mybir.dt.float32
    bf16 = mybir.dt.bfloat16
    fout = bf16 if OUT_BF16 else f32

    nc = bacc.Bacc("TRN2", target_bir_lowering=False, debug=False,
                   num_devices=NCORES)

    xe = nc.dram_tensor("xe", [EPC, DIM, C], bf16, kind="ExternalInput")
    cb = nc.dram_tensor("cb", [EPC, 128, C], f32, kind="ExternalInput")
    we1 = nc.dram_tensor("we1", [EPC, DIM, HID], bf16, kind="ExternalInput")
    we3 = nc.dram_tensor("we3", [EPC, DIM, HID], bf16, kind="ExternalInput")
    we2 = nc.dram_tensor("we2", [EPC, HID, DIM], bf16, kind="ExternalInput")
    xs = nc.dram_tensor("xs", [DIM, S], bf16, kind="ExternalInput")
    ws1 = nc.dram_tensor("ws1", [DIM, HID], bf16, kind="ExternalInput")
    ws3 = nc.dram_tensor("ws3", [DIM, HID], bf16, kind="ExternalInput")
    ws2 = nc.dram_tensor("ws2", [HID, DIM], bf16, kind="ExternalInput")
    oute = nc.dram_tensor("oute", [EPC, DIM, C], fout, kind="ExternalOutput")
    outs = nc.dram_tensor("outs", [DIM, S], fout, kind="ExternalOutput")

    DK = DIM // 128   # 8 contraction tiles for the up-projections
    HK = HID // 128   # 4 contraction tiles for the down-projection
    KH = DK // 2

    def as_pkf(ap):
        return ap.rearrange("(k p) f -> p k f", p=128)

    with tile.TileContext(nc) as tc:
        with (
            tc.tile_pool(name="wts", bufs=1) as wts,
            tc.tile_pool(name="acts", bufs=1) as actp,
            tc.tile_pool(name="work", bufs=2) as work,
            tc.tile_pool(name="ost", bufs=2) as ostp,
            tc.tile_pool(name="ph", bufs=2, space="PSUM") as ph,
            tc.tile_pool(name="po", bufs=2, space="PSUM") as po,
        ):
            jobs = []
            # expert 0: first-needed tensors in k-halves for a fast start
            w1h = [wts.tile([128, KH, HID], bf16, name=f"w1_0{h}")
                   for h in range(2)]
            w3h = [wts.tile([128, KH, HID], bf16, name=f"w3_0{h}")
                   for h in range(2)]
            xeh = [actp.tile([128, KH, C], bf16, name=f"xe_0{h}")
                   for h in range(2)]
            cb0 = actp.tile([128, C], f32, name="cbt_0")
            w20 = wts.tile([128, HK, DIM], bf16, name="w2_0")
            nc.sync.dma_start(out=w1h[0][:],
                              in_=as_pkf(we1[0])[:, 0:KH, :])
            nc.sync.dma_start(out=xeh[0][:],
                              in_=as_pkf(xe[0])[:, 0:KH, :])
            nc.sync.dma_start(out=w1h[1][:],
                              in_=as_pkf(we1[0])[:, KH:DK, :])
            nc.sync.dma_start(out=xeh[1][:],
                              in_=as_pkf(xe[0])[:, KH:DK, :])
            nc.sync.dma_start(out=w3h[0][:],
                              in_=as_pkf(we3[0])[:, 0:KH, :])
            nc.sync.dma_start(out=w3h[1][:],
                              in_=as_pkf(we3[0])[:, KH:DK, :])
            nc.sync.dma_start(out=cb0[:], in_=cb[0])
            nc.sync.dma_start(out=w20[:], in_=as_pkf(we2[0]))

            def half_slices(tiles):
                def sl(k, csl):
                    return tiles[k // KH][:, k % KH, csl]
                return sl

            jobs.append((half_slices(w1h), half_slices(w3h),
                         lambda k, csl: w20[:, k, csl],
                         half_slices(xeh), cb0, as_pkf(oute[0]), C))

            for e in range(1, EPC):
                w1_t = wts.tile([128, DK, HID], bf16, name=f"w1_{e}")
                w3_t = wts.tile([128, DK, HID], bf16, name=f"w3_{e}")
                w2_t = wts.tile([128, HK, DIM], bf16, name=f"w2_{e}")
                x_t = actp.tile([128, DK, C], bf16, name=f"xe_{e}")
                cb_t = actp.tile([128, C], f32, name=f"cbt_{e}")
                nc.sync.dma_start(out=w1_t[:], in_=as_pkf(we1[e]))
                nc.sync.dma_start(out=x_t[:], in_=as_pkf(xe[e]))
                nc.sync.dma_start(out=w3_t[:], in_=as_pkf(we3[e]))
                nc.sync.dma_start(out=cb_t[:], in_=cb[e])
                nc.sync.dma_start(out=w2_t[:], in_=as_pkf(we2[e]))

                def mk(t):
                    return lambda k, csl: t[:, k, csl]
                jobs.append((mk(w1_t), mk(w3_t), mk(w2_t), mk(x_t), cb_t,
                             as_pkf(oute[e]), C))

            w1_s = wts.tile([128, DK, HID], bf16, name="sw1")
            w3_s = wts.tile([128, DK, HID], bf16, name="sw3")
            w2_s = wts.tile([128, HK, DIM], bf16, name="sw2")
            x_s = actp.tile([128, DK, S], bf16, name="xst")
            nc.sync.dma_start(out=w1_s[:], in_=as_pkf(ws1[:]))
            nc.sync.dma_start(out=x_s[:], in_=as_pkf(xs[:]))
            nc.sync.dma_start(out=w3_s[:], in_=as_pkf(ws3[:]))
            nc.sync.dma_start(out=w2_s[:], in_=as_pkf(ws2[:]))

            def mk(t):
                return lambda k, csl: t[:, k, csl]
            jobs.append((mk(w1_s), mk(w3_s), mk(w2_s), mk(x_s), None,
                         as_pkf(outs[:]), S))

            for (w1f, w3f, w2f, xf_, cb_t, o_ap, ntok) in jobs:
                for (n0, n) in _chunks(ntok):
                    csl = slice(n0, n0 + n)
                    act_t = []
                    for hm in range(HK):
                        hsl = slice(hm * 128, (hm + 1) * 128)
                        p1 = ph.tile([128, 512], f32, tag="h1", name="p1")
                        p3 = ph.tile([128, 512], f32, tag="h3", name="p3")
                        for k in range(DK):
                            nc.tensor.matmul(p1[:, :n], w1f(k, hsl),
                                             xf_(k, csl),
                                             start=(k == 0),
                                             stop=(k == DK - 1))
                        for k in range(DK):
                            nc.tensor.matmul(p3[:, :n], w3f(k, hsl),
                                             xf_(k, csl),
                                             start=(k == 0),
                                             stop=(k == DK - 1))
                        sil = work.tile([128, 512], bf16, tag="sil",
                                        name="sil")
                        nc.scalar.activation(sil[:, :n], p1[:, :n],
                                             mybir.ActivationFunctionType.Silu)
                        a = work.tile([128, 512], bf16, tag=f"act{hm}",
                                      name=f"act{hm}")
                        if cb_t is not None:
                            h3s = work.tile([128, 512], bf16, tag="h3s",
                                            name="h3s")
                            nc.vector.tensor_tensor(h3s[:, :n], p3[:, :n],
                                                    cb_t[:, csl],
                                                    mybir.AluOpType.mult)
                            nc.vector.tensor_tensor(a[:, :n], h3s[:, :n],
                                                    sil[:, :n],
                                                    mybir.AluOpType.mult)
                        else:
                            nc.vector.tensor_tensor(a[:, :n], p3[:, :n],
                                                    sil[:, :n],
                                                    mybir.AluOpType.mult)
                        act_t.append(a)
                    stage = ostp.tile([128, DK, 512], fout, tag="stage",
                                      name="stage")
                    for dm in range(DK):
                        dsl = slice(dm * 128, (dm + 1) * 128)
                        pout = po.tile([128, 512], f32, tag="o", name="pout")
                        for k in range(HK):
                            nc.tensor.matmul(pout[:, :n], w2f(k, dsl),
                                             act_t[k][:, :n],
                                             start=(k == 0),
                                             stop=(k == HK - 1))
                        nc.vector.tensor_copy(out=stage[:, dm, :n],
                                              in_=pout[:, :n])
                    nc.sync.dma_start(out=o_ap[:, :, csl],
                                      in_=stage[:, :, :n])

    nc.compile()
    return nc


def _get_nc(C: int):
    if C not in _CACHE:
        _CACHE[C] = _build(C)
    return _CACHE[C]


LAST_RESULTS = None  # BassKernelResults from the most recent run (for test.py)


def kernel(x, gate_w, w1, w3, w2, sw1, sw3, sw2):
    global LAST_RESULTS
    from concourse.bass_utils import run_bass_kernel_spmd

    x = np.asarray(x)
    xf = np.ascontiguousarray(x.reshape(-1, DIM).astype(np.float32))
    gate_w = np.asarray(gate_w, dtype=np.float32)

    # ---- router on host (softmax -> top-4 -> renormalize) ----
    logits = xf @ gate_w.T                      # [T, E]
    m = logits.max(axis=1, keepdims=True)
    p = np.exp(logits - m)
    probs = p / p.sum(axis=1, keepdims=True)
    idx4 = np.argpartition(-probs, TOPK, axis=1)[:, :TOPK]     # [T, 4]
    w4 = np.take_along_axis(probs, idx4, axis=1)
    w4 = w4 / w4.sum(axis=1, keepdims=True)

    rows = np.repeat(np.arange(xf.shape[0]), TOPK)
    cols = idx4.ravel()
    vals = w4.ravel()

    tok_of = [rows[cols == e] for e in range(E)]
    cw_of = [vals[cols == e].astype(np.float32) for e in range(E)]
    counts = np.array([len(t) for t in tok_of])
    C = int(max(512, -(-counts.max() // 64) * 64))

    xf_bf = xf.astype(BF16)
    w1 = np.asarray(w1, dtype=np.float32)
    w3 = np.asarray(w3, dtype=np.float32)
    w2 = np.asarray(w2, dtype=np.float32)

    in_maps = []
    for c in range(NCORES):
        es = [c * EPC + j for j in range(EPC)]
        xe_np = np.zeros((EPC, DIM, C), dtype=BF16)
        cb_np = np.zeros((EPC, 128, C), dtype=np.float32)
        for j, e in enumerate(es):
            cnt = counts[e]
            xe_np[j, :, :cnt] = xf_bf[tok_of[e]].T
            cb_np[j, :, :cnt] = cw_of[e][None, :]
        im = {
            "xe": xe_np,
            "cb": cb_np,
            "we1": np.ascontiguousarray(
                w1[es].transpose(0, 2, 1)).astype(BF16),
            "we3": np.ascontiguousarray(
                w3[es].transpose(0, 2, 1)).astype(BF16),
            "we2": np.ascontiguousarray(
                w2[es].transpose(0, 2, 1)).astype(BF16),
            "xs": np.ascontiguousarray(xf_bf[c * S:(c + 1) * S].T),
            "ws1": np.ascontiguousarray(np.asarray(sw1, np.float32).T).astype(BF16),
            "ws3": np.ascontiguousarray(np.asarray(sw3, np.float32).T).astype(BF16),
            "ws2": np.ascontiguousarray(np.asarray(sw2, np.float32).T).astype(BF16),
        }
        in_maps.append(im)

    nc = _get_nc(C)
    trace = os.environ.get("KERNEL_TRACE", "0") == "1"
    res = run_bass_kernel_spmd(nc, in_maps, core_ids=list(range(NCORES)),
                               trace=trace)
    LAST_RESULTS = res

    out = np.zeros((T, DIM), dtype=np.float32)
    for c in range(NCORES):
        r = res.results[c]
        for j in range(EPC):
            e = c * EPC + j
            cnt = counts[e]
            out[tok_of[e]] += r["oute"][j, :, :cnt].T.astype(np.float32)
        out[c * S:(c + 1) * S] += r["outs"].T.astype(np.float32)
    return out.reshape(x.shape).astype(np.float32)
